# revision 17
# baseline (speedup 1.0000x reference)
"""Trainium2 Bass kernel: 2-layer GraphConv (PyG, aggr='add') on 8 NeuronCores.

  h   = relu(segsum(x[src]->dst) @ W1_rel.T + x @ W1_root.T + b1)
  out =      segsum(h[src]->dst) @ W2_rel.T + h @ W2_root.T + b2

Sharding: dst nodes contiguously across 8 cores (5000/core); edges
partitioned by dst core. Per core, edges are grouped by 128-node dst block
and by src half (lo/hi at node 20000 — dma_gather indices are int16), each
(block, half) padded to whole 128-edge tiles with tile counts shared across
cores so the single SPMD program is uniform; per-core variation is data only.

Per 128-edge tile: dma_gather pulls the bf16 source rows ([128 e, 128 ch],
edge on partition), VectorE builds a one-hot S tile ([128 e, 128 dstlocal]
via is_equal against an iota row), and TensorE computes
psum[ch, dst] += G.T-contract-S, accumulating a dst block's aggregate
transposed in PSUM. Dense projections use pre-transposed weights as
stationary operands on the aggT / xT column blocks, fusing bias+relu into
the ScalarE PSUM evacuation, producing h^T directly. h is transposed back
to rows per block (PE transpose) and AllGather'd across cores for the
layer-2 gather. The output is returned as [64, 5000] per core and
reassembled (transpose + concat) on the host.
"""
import sys
import numpy as np

if '/opt/trn_rl_repo' not in sys.path:
    sys.path.insert(0, '/opt/trn_rl_repo')

import concourse.bass as bass
import concourse.bacc as bacc
import concourse.tile as tile
import concourse.mybir as mybir
from concourse.bass_utils import run_bass_kernel_spmd
from concourse.bass_interp import get_hw_module

BF16 = mybir.dt.np(mybir.dt.bfloat16)
F32 = mybir.dt.float32
BF = mybir.dt.bfloat16
I16 = mybir.dt.int16


class Cfg:
    def __init__(self, N=40000, C=128, O=64, R=8, CHUNK=64, SGRP=8):
        self.N, self.C, self.O, self.R = N, C, O, R
        self.NPC = N // R
        self.BLK = 128
        self.NB = -(-self.NPC // self.BLK)
        self.SPLIT = N // 2
        self.CHUNK = CHUNK      # tiles per dma_gather call
        self.SGRP = SGRP        # tiles per S-build VectorE op
        self.PAD_DL = 30000.0   # dst_local pad value (!= any 0..127)


def schedule(cfg, edge_index):
    """Shared tile counts + per-core gather/dstloc data.

    Returns (T [NB,2] int, cores: list of dicts with idx0/idx1 [128,NT*8] i16
    and dstloc [128,Ttot] bf16 in matmul order).
    """
    src = np.asarray(edge_index[0], np.int64)
    dst = np.asarray(edge_index[1], np.int64)
    core = dst // cfg.NPC
    dloc = dst - core * cfg.NPC
    blk = dloc // cfg.BLK
    half = (src >= cfg.SPLIT).astype(np.int64)

    counts = np.zeros((cfg.R, cfg.NB, 2), np.int64)
    np.add.at(counts, (core, blk, half), 1)
    T = (-(-counts // cfg.BLK)).max(axis=0)  # [NB, 2]
    NT = T.sum(axis=0)                       # tiles per stream

    # order within a core: sort edges by (half, blk) keeping stable order
    cores = []
    for r in range(cfg.R):
        m = core == r
        s_r, dl_r, b_r, h_r = src[m], dloc[m], blk[m], half[m]
        idx_streams = []
        dl_cols = [[None, None] for _ in range(cfg.NB)]  # per block per half
        for s in (0, 1):
            parts = []
            for b in range(cfg.NB):
                mb = (b_r == b) & (h_r == s)
                si = s_r[mb] - (cfg.SPLIT if s else 0)
                di = dl_r[mb] - b * cfg.BLK
                pad = int(T[b, s]) * cfg.BLK - len(si)
                parts.append(np.concatenate([si, np.zeros(pad, np.int64)]))
                dl_cols[b][s] = np.concatenate(
                    [di, np.full(pad, cfg.PAD_DL, np.float64)])
            stream = np.concatenate(parts).astype(np.int16)
            wrapped = np.tile(stream.reshape(-1, 16).T, (8, 1))  # [128, NT*8]
            idx_streams.append(np.ascontiguousarray(wrapped))
        # dstloc in matmul order: for b: lo tiles then hi tiles
        cols = []
        for b in range(cfg.NB):
            for s in (0, 1):
                cols.append(dl_cols[b][s].reshape(-1, cfg.BLK))
        dstloc = np.concatenate(cols, axis=0).T  # [BLK, Ttot]
        cores.append(dict(idx0=idx_streams[0], idx1=idx_streams[1],
                          dstloc=np.ascontiguousarray(dstloc).astype(BF16)))
    return T, NT, cores


def build(cfg, T, NT):
    """Build + compile the SPMD Bass program (schedule-shaped)."""
    N, C, O, BLK, NB, NPC = cfg.N, cfg.C, cfg.O, cfg.BLK, cfg.NB, cfg.NPC
    Ttot = int(T.sum())
    nc = bacc.Bacc("TRN2", target_bir_lowering=False, debug=False,
                   num_devices=cfg.R, num_swdge_queues=4)

    xrows = nc.dram_tensor("xrows", [N, C], BF, kind="ExternalInput")
    xT = nc.dram_tensor("xT", [C, NPC], BF, kind="ExternalInput")
    idx0 = nc.dram_tensor("idx0", [128, int(NT[0]) * 8], I16, kind="ExternalInput")
    idx1 = nc.dram_tensor("idx1", [128, int(NT[1]) * 8], I16, kind="ExternalInput")
    dstloc = nc.dram_tensor("dstloc", [BLK, Ttot], BF, kind="ExternalInput")
    iota = nc.dram_tensor("iota", [128, cfg.SGRP * BLK], BF, kind="ExternalInput")
    ident = nc.dram_tensor("ident", [128, 128], BF, kind="ExternalInput")
    w1relT = nc.dram_tensor("w1relT", [C, C], BF, kind="ExternalInput")
    w1rootT = nc.dram_tensor("w1rootT", [C, C], BF, kind="ExternalInput")
    w2relT = nc.dram_tensor("w2relT", [C, O], BF, kind="ExternalInput")
    w2rootT = nc.dram_tensor("w2rootT", [C, O], BF, kind="ExternalInput")
    b1v = nc.dram_tensor("b1v", [C, 1], F32, kind="ExternalInput")
    b2v = nc.dram_tensor("b2v", [O, 1], F32, kind="ExternalInput")
    outT = nc.dram_tensor("outT", [O, NPC], F32, kind="ExternalOutput")

    with tile.TileContext(nc) as tc:
        from contextlib import ExitStack
        with ExitStack() as ctx:
            ep = ctx.enter_context
            const_p = ep(tc.tile_pool(name="const", bufs=1))
            g0_p = ep(tc.tile_pool(name="g0", bufs=3))
            g1_p = ep(tc.tile_pool(name="g1", bufs=3))
            s_p = ep(tc.tile_pool(name="spool", bufs=8))
            aggT_p = ep(tc.tile_pool(name="aggTsb", bufs=3))
            hrow_p = ep(tc.tile_pool(name="hrow", bufs=2))
            paggT_p = ep(tc.tile_pool(name="paggT", bufs=3, space="PSUM"))
            ph_p = ep(tc.tile_pool(name="ph", bufs=2, space="PSUM"))
            ptr_p = ep(tc.tile_pool(name="ptr", bufs=2, space="PSUM"))
            dram_p = ep(tc.tile_pool(name="dram", bufs=1, space="DRAM"))

            def load(dram_t, shape, dtype, tag):
                t = const_p.tile(shape, dtype, tag=tag)
                nc.sync.dma_start(t[:], dram_t[:, :])
                return t

            idx_sb = [load(idx0, [128, int(NT[0]) * 8], I16, "idx0"),
                      load(idx1, [128, int(NT[1]) * 8], I16, "idx1")]
            dstloc_sb = load(dstloc, [BLK, Ttot], BF, "dstloc")
            iota_sb = load(iota, [128, cfg.SGRP * BLK], BF, "iota")
            ident_sb = load(ident, [128, 128], BF, "ident")
            w1relT_sb = load(w1relT, [C, C], BF, "w1relT")
            w1rootT_sb = load(w1rootT, [C, C], BF, "w1rootT")
            w2relT_sb = load(w2relT, [C, O], BF, "w2relT")
            w2rootT_sb = load(w2rootT, [C, O], BF, "w2rootT")
            b1_sb = load(b1v, [C, 1], F32, "b1v")
            b2_sb = load(b2v, [O, 1], F32, "b2v")
            xT_sb = load(xT, [C, NPC], BF, "xT")
            hT_sb = const_p.tile([C, NPC], BF)
            outT_sb = const_p.tile([O, NPC], F32)

            h_loc = dram_p.tile([NPC, C], BF)
            h_full = dram_p.tile([N, C], BF)

            def emit_dense(layer, b, wb, psum_aggT):
                c0 = b * BLK
                aggT_sb = aggT_p.tile([C, BLK], BF)
                nc.scalar.copy(aggT_sb[:], psum_aggT[:])
                if layer == 0:
                    ph = ph_p.tile([C, BLK], F32, tag="ph")
                    nc.tensor.matmul(ph[:, :wb], lhsT=w1relT_sb[:],
                                     rhs=aggT_sb[:, :wb], start=True, stop=False)
                    nc.tensor.matmul(ph[:, :wb], lhsT=w1rootT_sb[:],
                                     rhs=xT_sb[:, c0:c0 + wb], start=False, stop=True)
                    nc.scalar.activation(hT_sb[:, c0:c0 + wb], ph[:, :wb],
                                         mybir.ActivationFunctionType.Relu,
                                         bias=b1_sb[:, :1])
                    ptr = ptr_p.tile([128, 128], BF)
                    nc.tensor.transpose(ptr[:wb, :], hT_sb[:, c0:c0 + wb],
                                        ident_sb[:])
                    hrow = hrow_p.tile([128, C], BF)
                    nc.vector.tensor_copy(hrow[:wb, :], ptr[:wb, :])
                    nc.sync.dma_start(h_loc[c0:c0 + wb, :], hrow[:wb, :])
                else:
                    po = ph_p.tile([C, BLK], F32, tag="ph")
                    nc.tensor.matmul(po[:O, :wb], lhsT=w2relT_sb[:],
                                     rhs=aggT_sb[:, :wb], start=True, stop=False)
                    nc.tensor.matmul(po[:O, :wb], lhsT=w2rootT_sb[:],
                                     rhs=hT_sb[:, c0:c0 + wb], start=False, stop=True)
                    nc.scalar.activation(outT_sb[:, c0:c0 + wb], po[:O, :wb],
                                         mybir.ActivationFunctionType.Identity,
                                         bias=b2_sb[:, :1])

            def emit_layer(layer):
                if layer == 0:
                    srcs = [xrows[0:cfg.SPLIT, :], xrows[cfg.SPLIT:N, :]]
                else:
                    srcs = [h_full[0:cfg.SPLIT, :], h_full[cfg.SPLIT:N, :]]
                g_pool = [g0_p, g1_p]
                chunk_tiles = [dict(), dict()]
                sgroups = dict()
                jmm = 0
                q_base = [0, 0]
                pending = None
                qrr = [0]  # SWDGE queue round-robin across Q7 core pairs
                for b in range(NB):
                    wb = min(BLK, NPC - b * BLK)
                    psum_aggT = paggT_p.tile([C, BLK], F32)
                    mms = [(s, q_base[s] + t)
                           for s in (0, 1) for t in range(int(T[b, s]))]
                    assert mms, f"block {b} has no edge tiles"
                    for i, (s, q) in enumerate(mms):
                        ci = q // cfg.CHUNK
                        if ci not in chunk_tiles[s]:
                            ntc = min(cfg.CHUNK, int(NT[s]) - ci * cfg.CHUNK)
                            gt = g_pool[s].tile([128, cfg.CHUNK * C], BF)
                            nidx = ntc * 128
                            nc.gpsimd.dma_gather(
                                gt[:, :ntc * C].rearrange("p (t e) -> p t e", e=C),
                                srcs[s],
                                idx_sb[s][:, ci * cfg.CHUNK * 8:
                                          ci * cfg.CHUNK * 8 + ntc * 8],
                                nidx, nidx, C, elem_step=C,
                                single_packet=False,
                                queue_num=qrr[0] % 4)
                            qrr[0] += 1
                            chunk_tiles[s][ci] = gt
                        gi = jmm // cfg.SGRP
                        if gi not in sgroups:
                            gw = min(cfg.SGRP, Ttot - gi * cfg.SGRP)
                            st = s_p.tile([128, cfg.SGRP * BLK], BF, tag="st")
                            nc.vector.tensor_tensor(
                                out=st[:, :gw * BLK].rearrange(
                                    "p (t e) -> p t e", e=BLK),
                                in0=iota_sb[:, :gw * BLK].rearrange(
                                    "p (t e) -> p t e", e=BLK),
                                in1=dstloc_sb[:, gi * cfg.SGRP:gi * cfg.SGRP + gw]
                                    .to_broadcast([128, gw, BLK]),
                                op=mybir.AluOpType.is_equal)
                            sgroups[gi] = st
                        qs = q - ci * cfg.CHUNK
                        nc.tensor.matmul(
                            psum_aggT[:],
                            lhsT=chunk_tiles[s][ci][:, qs * C:(qs + 1) * C],
                            rhs=sgroups[gi][:, (jmm % cfg.SGRP) * BLK:
                                            (jmm % cfg.SGRP) * BLK + BLK],
                            start=(i == 0), stop=(i == len(mms) - 1))
                        jmm += 1
                    for s in (0, 1):
                        q_base[s] += int(T[b, s])
                    if pending is not None:
                        emit_dense(layer, *pending)
                    pending = (b, wb, psum_aggT)
                emit_dense(layer, *pending)

            emit_layer(0)
            nc.gpsimd.collective_compute(
                "AllGather", mybir.AluOpType.bypass,
                replica_groups=[list(range(cfg.R))],
                ins=[h_loc.opt()], outs=[h_full.opt()])
            emit_layer(1)
            nc.sync.dma_start(outT[:, :], outT_sb[:, :])

    nc.compile()
    return nc


def make_in_maps(cfg, inputs, T, NT, cores):
    x = np.asarray(inputs['x'], np.float32)
    x_bf = x.astype(BF16)
    W1_rel = np.asarray(inputs['W1_rel'], np.float32)
    W1_root = np.asarray(inputs['W1_root'], np.float32)
    W2_rel = np.asarray(inputs['W2_rel'], np.float32)
    W2_root = np.asarray(inputs['W2_root'], np.float32)
    b1 = np.asarray(inputs['b1'], np.float32).reshape(cfg.C, 1)
    b2 = np.asarray(inputs['b2'], np.float32).reshape(cfg.O, 1)

    shared = dict(
        xrows=x_bf,
        iota=np.tile(np.arange(cfg.BLK, dtype=np.float32),
                     cfg.SGRP).reshape(1, -1).repeat(128, 0).astype(BF16),
        ident=np.eye(128, dtype=np.float32).astype(BF16),
        w1relT=np.ascontiguousarray(W1_rel.T).astype(BF16),
        w1rootT=np.ascontiguousarray(W1_root.T).astype(BF16),
        w2relT=np.ascontiguousarray(W2_rel.T).astype(BF16),
        w2rootT=np.ascontiguousarray(W2_root.T).astype(BF16),
        b1v=b1, b2v=b2,
    )
    in_maps = []
    for r in range(cfg.R):
        cr = cores[r]
        in_maps.append(dict(
            shared,
            xT=np.ascontiguousarray(
                x_bf[r * cfg.NPC:(r + 1) * cfg.NPC, :].T),
            idx0=cr['idx0'], idx1=cr['idx1'], dstloc=cr['dstloc'],
        ))
    return in_maps


def run(cfg, nc, in_maps, trace=False, **kw):
    nc_run = nc
    res = run_bass_kernel_spmd(nc_run, in_maps,
                               core_ids=list(range(cfg.R)), trace=trace, **kw)
    out = np.concatenate(
        [np.asarray(res.results[r]["outT"], np.float32).T
         for r in range(cfg.R)], axis=0)
    return out, res


def kernel(**inputs):
    cfg = Cfg()
    T, NT, cores = schedule(cfg, np.asarray(inputs['edge_index']))
    nc = build(cfg, T, NT)
    nc.m = get_hw_module(nc.m)
    in_maps = make_in_maps(cfg, inputs, T, NT, cores)
    out, _ = run(cfg, nc, in_maps)
    return out.astype(np.float32)
